# revision 19
# baseline (speedup 1.0000x reference)
"""ArcFace logits kernel for 8 TRN2 NeuronCores (partial-FC tensor parallel).

logits = scale * where(one_hot(labels), cos(arccos(cosine)+m), cosine)
  cosine = normalize(emb) @ normalize(W)   [B=512, C=100000]

Sharding: W columns (and the [B, C] output) split across 8 cores, 12500
columns each; embeddings/labels broadcast. No collectives needed.

I/O precision: W (and emb/W[:,labels]) staged to device HBM as bf16 (the
PE consumes bf16 anyway); output written bf16 and upcast to f32 on
gather — halves both big HBM streams. All math (norms, margin chain,
matmul) runs on device.

Broadcast-free reductions: every partition-axis reduce uses an all-ones
[128,128] stationary operand, so the [*, N] psum result lands REPLICATED
across all 128 partitions — same matmul cycles (cost is N streaming
columns), but downstream per-column scales need no broadcast matmul,
no psum eviction copy, and no extra psum bank.

Engine assignment per 500-column chunk (25 chunks, no padding):
  - PE:  16 main matmuls (psum = (64*emb_norm)^T @ W_bf16 over D) +
         1 row-packed one-hot margin-fix matmul (K<=32 at partition
         offsets 0/32/64/96 via tile_position) + 2 fp8 DoubleRow
         ones-matmuls for the column norms (K=256 each, M=128)
  - ACT: all 4 squares as ONE 3D Square op -> fp8 pair tile, plus
         Ln+Exp for rsqrt of the norms (on the replicated [128, 500])
  - DVE: the 4 psum evicts (out_bf16 = psum * rsqrt_replicated)
  - GPS: sel/oh mask DMAs + one multiply building the fix lhsT
         (sel_fp8 * margin_delta_bf16)
  - SYNC ring: W in + out writes (scalar ring carries the prologue
    loads and warms up first)

The margin delta is computed on device from W[:, labels]:
  d64pre = (64(cos m - 1) u - 64 sin(m) sqrt(|w|^2 - u^2)) , u = <e,w>/|e|
(pre-multiplied by |w_label| so the evict's 1/|w| cancels); host passes
only 0/1 selection/one-hot masks (pure label-index plumbing),
pre-replicated across the 4 row-pack offsets.
"""

import math

import numpy as np

import concourse.bass as bass
import concourse.tile as tile
from concourse import mybir
from concourse.bass_utils import run_bass_kernel_spmd

N_CORES = 8
B = 512          # batch
D = 512          # embed dim
C = 100000       # num classes
CS = C // N_CORES          # 12500 columns per core
CHUNK = 500
NCHUNK = CS // CHUNK       # 25
DT = D // 128
BT = B // 128
SCALE = 64.0
MARGIN = 0.5
F32 = mybir.dt.float32
BF16 = mybir.dt.bfloat16
FP8 = mybir.dt.float8e4
AF = mybir.ActivationFunctionType

SQPAD = 512        # Ko-pair stride for DoubleRow rhs (16B-aligned)

_MAX_WAITS = 1


def _legalize_waits(nc, max_waits=_MAX_WAITS):
    """Split multi-wait instructions for this toolchain's codegen.

    The pinned neuronxcc rejects instructions carrying more than one sync
    wait ("Too many sync wait commands" in setupSyncWait). Tile's semaphore
    assignment can attach several waits to one instruction (tail drain,
    first matmul of a group). Hoist the overflow onto no-op instructions
    emitted just before, on the same engine — the engine blocks on those
    first, which is semantically identical.
    """
    n = 0
    for fn in nc.m.functions:
        for bb in fn.blocks:
            out = []
            for inst in bb.instructions:
                si = inst.sync_info
                if si is not None and si.on_wait and len(si.on_wait) > max_waits:
                    waits = list(si.on_wait)
                    keep = waits[-max_waits:]
                    over = waits[:-max_waits]
                    for i in range(0, len(over), max_waits):
                        nop = mybir.InstNoOp(
                            name=f"waitsplit_{n}",
                            sync_info=mybir.SyncInfo(
                                on_wait=over[i : i + max_waits], on_update=[]
                            ),
                            bass_nofuse=True,
                            engine=inst.engine,
                        )
                        n += 1
                        nc.register_instruction(nop)
                        out.append(nop)
                    inst.sync_info = mybir.SyncInfo(
                        on_wait=keep, on_update=list(si.on_update or [])
                    )
                out.append(inst)
            bb.instructions[:] = out
    return n


def build(k_fix, sq_bufs=4, w_bufs=5, out_bufs=4, ps_bufs=7):
    assert k_fix <= 32
    nchunk = NCHUNK
    nc = bass.Bass("TRN2", target_bir_lowering=False, debug=False, num_devices=N_CORES)
    w_ext = nc.declare_dram_parameter("w", [D, CS], BF16, isOutput=False)
    embt_ext = nc.declare_dram_parameter("embT", [D, B], BF16, isOutput=False)
    wl_ext = nc.declare_dram_parameter("wl", [D, B], BF16, isOutput=False)
    sel_ext = nc.declare_dram_parameter("sel", [nchunk, 128, B], FP8, isOutput=False)
    oh_ext = nc.declare_dram_parameter("oh", [nchunk, 128, CHUNK], BF16, isOutput=False)
    ones_ext = nc.declare_dram_parameter("ones", [128, 128], BF16, isOutput=False)
    ones8_ext = nc.declare_dram_parameter("ones8", [128, 2, 128], FP8, isOutput=False)
    out_ext = nc.declare_dram_parameter("out", [B, CS], BF16, isOutput=True)

    w_ap = w_ext.ap().rearrange("(a p) c -> p a c", p=128)      # [128, DT, cs]
    et_ap = embt_ext.ap().rearrange("(a p) b -> p a b", p=128)  # [128, DT, B]
    wl_ap = wl_ext.ap().rearrange("(a p) b -> p a b", p=128)
    out_ap = out_ext.ap().rearrange("(a p) c -> p a c", p=128)  # [128, BT, cs]

    cosm = math.cos(MARGIN)
    sinm = math.sin(MARGIN)

    with tile.TileContext(nc) as tc:
        with (
            tc.tile_pool(name="persist", bufs=1) as persist,
            tc.tile_pool(name="ps", bufs=ps_bufs, space="PSUM") as psp,
            tc.tile_pool(name="pro", bufs=3) as pro,
            tc.tile_pool(name="wp", bufs=w_bufs) as wp,
            tc.tile_pool(name="sqp", bufs=sq_bufs) as sqp,
            tc.tile_pool(name="rp", bufs=3) as rp,
            tc.tile_pool(name="op", bufs=out_bufs) as op,
            tc.tile_pool(name="fx", bufs=3) as fx,
        ):
            # ---- PE warmup: dummy matmuls on memset data fill the
            # DMA-wait window at kernel start so HAM reaches full clock
            # before the first real matmul arrives.
            warm = persist.tile([128, 512], BF16)
            nc.vector.memset(warm[:], 1.0)
            p_warm = psp.tile([128, 512], F32, tag="psn", bufs=1)
            for i in range(10):
                nc.tensor.matmul(
                    p_warm[:], lhsT=warm[:, 0:128], rhs=warm[:],
                    start=(i == 0), stop=(i == 9),
                )

            # ---- persistent tiles (small DMAs first on each ring)
            ones = persist.tile([128, 128], BF16)
            nc.sync.dma_start(out=ones[:], in_=ones_ext.ap())
            ones8 = persist.tile([128, 2, 128], FP8)
            nc.sync.dma_start(out=ones8[:], in_=ones8_ext.ap())

            wb_t = {}
            sq_t = {}
            rb_t = {}
            fx_t = {}

            def stage_a0(j):
                wb = wp.tile([128, DT, CHUNK], BF16, tag="wb")
                nc.sync.dma_start(
                    out=wb[:], in_=w_ap[:, :, j * CHUNK : (j + 1) * CHUNK]
                )
                wb_t[j] = wb

            def stage_sel(j):
                selt = fx.tile([128, B], FP8, tag="selt")
                nc.gpsimd.dma_start(out=selt[:], in_=sel_ext.ap()[j])
                oht = fx.tile([128, CHUNK], BF16, tag="oht")
                nc.gpsimd.dma_start(out=oht[:], in_=oh_ext.ap()[j])
                fx_t[j] = (selt, oht)

            def stage_a1(j, dve=False):
                wb = wb_t[j]
                sqq = sqp.tile([128, DT, SQPAD], FP8, tag="sq")
                if dve:
                    nc.vector.tensor_mul(sqq[:, :, :CHUNK], wb[:], wb[:])
                else:
                    nc.scalar.activation(
                        out=sqq[:, :, :CHUNK], in_=wb[:], func=AF.Square
                    )
                sq_t[j] = sqq

            def stage_a2_mm(j):
                # norm reduce, replicated to all partitions (M=128 ones)
                sqq = sq_t.pop(j)
                p_n = psp.tile([128, CHUNK], F32, tag="psn", bufs=1)
                nc.tensor.matmul(
                    p_n[:], lhsT=ones8[:], rhs=sqq[:, 0:2, :CHUNK],
                    start=True, stop=False,
                    perf_mode=mybir.MatmulPerfMode.DoubleRow,
                )
                nc.tensor.matmul(
                    p_n[:], lhsT=ones8[:], rhs=sqq[:, 2:4, :CHUNK],
                    start=False, stop=True,
                    perf_mode=mybir.MatmulPerfMode.DoubleRow,
                )
                return p_n

            def stage_a2_act(j, p_n):
                lnn = rp.tile([128, CHUNK], F32, tag="lnn")
                nc.scalar.activation(out=lnn[:], in_=p_n[:], func=AF.Ln)
                rb = rp.tile([128, CHUNK], BF16, tag="rb")
                nc.scalar.activation(out=rb[:], in_=lnn[:], func=AF.Exp, scale=-0.5)
                rb_t[j] = rb

            def stage_a2(j):
                stage_a2_act(j, stage_a2_mm(j))

            # ---- pre-roll. The scalar HWDGE ring starts moving bytes
            # several us before the sync ring does, so the prologue-critical
            # loads (W0, emb, W[:,labels], W1) go there in priority order;
            # W2/W3 warm up the sync ring in parallel.
            def scalar_w(jj):
                wb = wp.tile([128, DT, CHUNK], BF16, tag="wb")
                nc.scalar.dma_start(
                    out=wb[:], in_=w_ap[:, :, jj * CHUNK : (jj + 1) * CHUNK]
                )
                wb_t[jj] = wb

            scalar_w(0)
            et = persist.tile([128, DT, B], BF16)
            nc.scalar.dma_start(out=et[:], in_=et_ap)
            wlt = pro.tile([128, DT, B], BF16, bufs=1)
            nc.scalar.dma_start(out=wlt[:], in_=wl_ap)
            scalar_w(1)
            stage_a0(2)
            stage_a0(3)
            stage_sel(0)
            stage_a1(0, dve=True)

            e64b = persist.tile([128, DT, B], BF16)
            dpb4 = persist.tile([128, B], BF16)

            # pA: rinv_e = 1/||emb|| per batch row, replicated on partitions
            p_ne = psp.tile([128, B], F32, tag="ps")
            for d in range(DT):
                sq = pro.tile([128, B], BF16, bufs=4)
                nc.scalar.activation(out=sq[:], in_=et[:, d, :], func=AF.Square)
                nc.tensor.matmul(
                    p_ne[:], lhsT=ones[:], rhs=sq[:], start=(d == 0), stop=(d == DT - 1)
                )
            lne = pro.tile([128, B], F32, bufs=1)
            nc.scalar.activation(out=lne[:], in_=p_ne[:], func=AF.Ln)
            rinv_e = pro.tile([128, B], F32, bufs=1)
            nc.scalar.activation(out=rinv_e[:], in_=lne[:], func=AF.Exp, scale=-0.5)
            ebc = pro.tile([128, B], F32, bufs=1)
            nc.scalar.mul(out=ebc[:], in_=rinv_e[:], mul=SCALE)

            stage_a2(0)

            # pB: dot = <e, w_label>, |w_label|^2, then the margin delta
            p_dot = psp.tile([128, B], F32, tag="ps")
            for d in range(DT):
                hd = pro.tile([128, B], BF16, bufs=4)
                nc.vector.tensor_mul(hd[:], et[:, d, :], wlt[:, d, :])
                nc.tensor.matmul(
                    p_dot[:], lhsT=ones[:], rhs=hd[:], start=(d == 0), stop=(d == DT - 1)
                )
            p_nw = psp.tile([128, B], F32, tag="ps")
            for d in range(DT):
                sqw = pro.tile([128, B], BF16, bufs=4)
                nc.vector.tensor_mul(sqw[:], wlt[:, d, :], wlt[:, d, :])
                nc.tensor.matmul(
                    p_nw[:], lhsT=ones[:], rhs=sqw[:], start=(d == 0), stop=(d == DT - 1)
                )
            for d in range(DT):
                nc.vector.tensor_mul(e64b[:, d, :], et[:, d, :], ebc[:])

            # u = dot/|e|;  d64pre = 64(cosm-1)u - 64 sinm sqrt(|w|^2-u^2)
            u = pro.tile([128, B], F32, bufs=1)
            nc.vector.tensor_mul(u[:], p_dot[:], rinv_e[:])
            u2 = pro.tile([128, B], F32)
            nc.vector.tensor_mul(u2[:], u[:], u[:])
            v = pro.tile([128, B], F32)
            nc.vector.tensor_sub(v[:], p_nw[:], u2[:])
            lnv = pro.tile([128, B], F32)
            nc.scalar.activation(out=lnv[:], in_=v[:], func=AF.Ln)
            s = pro.tile([128, B], F32)
            nc.scalar.activation(out=s[:], in_=lnv[:], func=AF.Exp, scale=0.5)
            t1 = pro.tile([128, B], F32)
            nc.vector.tensor_scalar_mul(t1[:], u[:], SCALE * (cosm - 1.0))
            t2 = pro.tile([128, B], F32)
            nc.vector.tensor_scalar_mul(t2[:], s[:], -SCALE * sinm)
            nc.vector.tensor_add(dpb4[:], t1[:], t2[:])

            stage_a1(1)
            stage_a2(1)
            stage_sel(1)
            stage_a1(2)
            stage_sel(2)

            # ---- main loop
            for j in range(nchunk):
                if j + 4 < nchunk:
                    stage_a0(j + 4)
                if j + 2 < nchunk:
                    stage_a2(j + 2)
                wb = wb_t.pop(j)
                rb = rb_t.pop(j)
                selt, oht = fx_t.pop(j)
                fixt = fx.tile([128, B], BF16, tag="fixt")
                nc.gpsimd.tensor_mul(fixt[:], selt[:], dpb4[:])

                outc = op.tile([128, BT, CHUNK], BF16, tag="outc")
                p_ms = []
                for bt in range(BT):
                    p_m = psp.tile([128, CHUNK], F32, tag="ps")
                    for d in range(DT):
                        nc.tensor.matmul(
                            p_m[:],
                            lhsT=e64b[:, d, bt * 128 : (bt + 1) * 128],
                            rhs=wb[:, d, :],
                            start=(d == 0),
                            stop=False,
                        )
                    p_ms.append(p_m)
                for bt in range(BT):
                    nc.tensor.matmul(
                        p_ms[bt][:],
                        lhsT=fixt[32 * bt : 32 * bt + k_fix, bt * 128 : (bt + 1) * 128],
                        rhs=oht[32 * bt : 32 * bt + k_fix, :],
                        start=False,
                        stop=True,
                        tile_position=(32 * bt, 0),
                    )
                for bt in range(BT):
                    nc.vector.tensor_mul(outc[:, bt, :], p_ms[bt][:], rb[:])
                if j + 2 < nchunk:
                    nc.sync.dma_start(
                        out=out_ap[:, :, j * CHUNK : (j + 1) * CHUNK], in_=outc[:]
                    )
                else:
                    # last chunks: ship per-bt so the tail drains sooner
                    for bt in range(BT):
                        nc.sync.dma_start(
                            out=out_ap[:, bt, j * CHUNK : (j + 1) * CHUNK],
                            in_=outc[:, bt, :],
                        )
                if j + 3 < nchunk:
                    stage_a1(j + 3)
                    stage_sel(j + 3)

    _legalize_waits(nc)
    return nc


def _host_prep(embeddings, labels, class_weights):
    embeddings = np.ascontiguousarray(np.asarray(embeddings, dtype=np.float32))
    labels = np.asarray(labels).astype(np.int64)
    class_weights = np.asarray(class_weights, dtype=np.float32)

    bf16 = mybir.dt.np(mybir.dt.bfloat16)
    fp8 = mybir.dt.np(mybir.dt.float8e4)
    embt = np.ascontiguousarray(embeddings.T.astype(bf16))           # [D, B]
    wl = np.ascontiguousarray(class_weights[:, labels].astype(bf16))  # [D, B]

    n_cores = class_weights.shape[1] // CS
    counts = np.zeros((n_cores, NCHUNK), dtype=np.int64)
    for l in labels:
        counts[l // CS, (l % CS) // CHUNK] += 1
    k_fix = max(8, int(counts.max()))
    assert k_fix <= 32, k_fix

    # selection/one-hot masks, replicated at partition offsets 0/32/64/96
    # so the per-chunk fix matmuls row-pack into the PE array
    sel = np.zeros((n_cores, NCHUNK, 128, B), dtype=fp8)
    oh = np.zeros((n_cores, NCHUNK, 128, CHUNK), dtype=bf16)
    slot = np.zeros((n_cores, NCHUNK), dtype=np.int64)
    for b, l in enumerate(labels):
        core = int(l) // CS
        j = (int(l) % CS) // CHUNK
        c_loc = (int(l) % CS) % CHUNK
        k = slot[core, j]
        slot[core, j] += 1
        for g in range(4):
            sel[core, j, 32 * g + k, b] = 1.0
            oh[core, j, 32 * g + k, c_loc] = 1.0

    ones = np.ones((128, 128), dtype=bf16)
    ones8 = np.ones((128, 2, 128), dtype=fp8)

    in_maps = []
    for core in range(n_cores):
        shard = class_weights[:, core * CS : (core + 1) * CS]
        in_maps.append(
            {
                "w": np.ascontiguousarray(shard.astype(bf16)),
                "embT": embt,
                "wl": wl,
                "sel": sel[core],
                "oh": oh[core],
                "ones": ones,
                "ones8": ones8,
            }
        )
    return k_fix, in_maps


def kernel(embeddings, labels, class_weights, _trace=False):
    k_fix, in_maps = _host_prep(embeddings, labels, class_weights)
    nc = build(k_fix)
    res = run_bass_kernel_spmd(
        nc, in_maps, core_ids=list(range(N_CORES)), trace=_trace
    )
    out = np.concatenate(
        [res.results[i]["out"] for i in range(N_CORES)], axis=1
    )
    if _trace:
        kernel.last_results = res
    return out.astype(np.float32)
